# revision 3
# baseline (speedup 1.0000x reference)
"""Trainium2 Bass kernel for the diffusion-sampler importance-weight problem.

Math (per batch element, per z-dim), derived from the reference:
  z_0 = sigma0 * eps0
  per step t (beta_f = beta[t], beta_b = roll(beta,1)[t]):
    hid   = relu(W1z @ z + c1 + te_t)          c1 = ctx @ W1[Z:] + b1 (host precomputed)
    u'    = W2' @ hid                           W2' = W2 * dt
    z'    = a_t * z + u' + s_t * eps_t          a_t = 1 + beta_f dt,  s_t = sqrt(2 beta_f dt) sig0
    logw += 0.5 eps_t^2 - k_t (z - c_t z')^2 + log(s_t / sb_t)
            c_t = 1 - beta_b dt, sb_t = sqrt(2 beta_b dt) sig0, k_t = 0.5 / sb_t^2
  terminal: logw += -0.5 (z_T - mu)^2 + 0.5 eps0^2 + log(sig0)
  output = sum_z mean_b logw

Device layout: feature-major activations — [Z=128 partitions, batch free].
Host pre-transposes eps/eps0/target_mu/c1, shards batch across 8 cores, and
builds per-step scalar tables. Matmuls run in float32r (tf32-grade, 1 cyc/row);
the trajectory-sensitive a_t*z and the d-term run exact fp32 on DVE.
"""
import math
import numpy as np

B, Z, H, HID, T = 16384, 128, 512, 512, 32
NCORES = 8
BS = B // NCORES          # 2048 batch rows per core
NT = BS // 512            # 4 n-tiles of 512
SQ5 = float(np.sqrt(0.5))

_cache: dict = {}
_ABLATE: set = set()
# tuning knobs (sim-swept)
KNOB = dict(dve_movers=(), eps_bufs=3, hid_bufs=6, scr_bufs=3, psh_bufs=5,
            psz_bufs=3, c1_dve=(), sqd_act=False)


def _build_module(nop=False, reps=1):
    import concourse.tile as tile
    from concourse import bacc, mybir

    f32 = mybir.dt.float32
    f32r = mybir.dt.float32r
    AF = mybir.ActivationFunctionType
    ALU = mybir.AluOpType

    nc = bacc.Bacc("TRN2", target_bir_lowering=False, debug=False,
                   num_devices=NCORES)

    epsT = nc.dram_tensor("epsT", [T, 128, BS], f32r, kind="ExternalInput").ap()
    eps0T = nc.dram_tensor("eps0T", [128, BS], f32, kind="ExternalInput").ap()
    muT = nc.dram_tensor("muT", [128, BS], f32, kind="ExternalInput").ap()
    c1d = nc.dram_tensor("c1d", [4, 128, BS], f32r, kind="ExternalInput").ap()
    w1zd = nc.dram_tensor("w1zd", [128, HID], f32r, kind="ExternalInput").ap()
    w2d = nc.dram_tensor("w2d", [HID, 128], f32r, kind="ExternalInput").ap()
    identd = nc.dram_tensor("identd", [128, 128], f32r, kind="ExternalInput").ap()
    diagd = nc.dram_tensor("diagd", [T, 128, 128], f32r, kind="ExternalInput").ap()
    tbld = nc.dram_tensor("tbld", [128, 160], f32, kind="ExternalInput").ap()
    tetd = nc.dram_tensor("tetd", [128, 128], f32, kind="ExternalInput").ap()
    outd = nc.dram_tensor("outd", [128, 2], f32, kind="ExternalOutput").ap()
    outd2 = nc.dram_tensor("outd2", [128, 2], f32, kind="ExternalOutput").ap()

    with tile.TileContext(nc) as tc:
        with (
            tc.tile_pool(name="const", bufs=1) as cpool,
            tc.tile_pool(name="state", bufs=1) as spool,
            tc.tile_pool(name="eps", bufs=KNOB["eps_bufs"]) as epool,
            tc.tile_pool(name="hid", bufs=KNOB["hid_bufs"]) as hpool,
            tc.tile_pool(name="scr", bufs=KNOB["scr_bufs"]) as scrp,
            tc.tile_pool(name="psH", bufs=KNOB["psh_bufs"], space="PSUM") as psH,
            tc.tile_pool(name="psZ", bufs=KNOB["psz_bufs"], space="PSUM") as psZ,
        ):
            if nop:
                out2 = spool.tile([128, 2], f32, tag="out2")
                nc.gpsimd.memset(out2[:], 0.0)
                nc.sync.dma_start(outd, out2[:])
                nc.sync.dma_start(outd2, out2[:])
            elif reps == 1:
                _emit(nc, tc, cpool, spool, epool, hpool, scrp, psH, psZ,
                      f32, f32r, AF, ALU,
                      epsT, eps0T, muT, c1d, w1zd, w2d, identd, diagd,
                      tbld, tetd, outd, outd2)
            else:
                with tc.For_i(0, reps, 1):
                    _emit(nc, tc, cpool, spool, epool, hpool, scrp, psH, psZ,
                          f32, f32r, AF, ALU,
                          epsT, eps0T, muT, c1d, w1zd, w2d, identd, diagd,
                          tbld, tetd, outd, outd2)

    nc.compile()
    return nc


def _emit(nc, tc, cpool, spool, epool, hpool, scrp, psH, psZ,
          f32, f32r, AF, ALU,
          epsT, eps0T, muT, c1d, w1zd, w2d, identd, diagd, tbld, tetd, outd,
          outd2):
    # ---- resident constants ----
    w1z = cpool.tile([128, HID], f32r, tag="w1z")
    nc.sync.dma_start(w1z[:], w1zd)
    w2t = cpool.tile([128, 4, 128], f32r, tag="w2t")
    nc.sync.dma_start(w2t[:], w2d.rearrange("(k p) m -> p k m", p=128))
    ident = cpool.tile([128, 128], f32r, tag="ident")
    nc.sync.dma_start(ident[:], identd)
    diag = cpool.tile([128, T, 128], f32r, tag="diag")
    nc.sync.dma_start(diag[:], diagd.rearrange("t p c -> p t c"))
    tbl = cpool.tile([128, 160], f32, tag="tbl")
    nc.sync.dma_start(tbl[:], tbld)
    tet = cpool.tile([128, 128], f32, tag="tet")
    nc.sync.dma_start(tet[:], tetd)
    c1t = cpool.tile([128, 4, BS], f32r, tag="c1t")
    nc.sync.dma_start(c1t[:], c1d.rearrange("h p b -> p h b"))

    zA = spool.tile([128, BS], f32, tag="zA")
    zB = spool.tile([128, BS], f32, tag="zB")
    slots_e0 = spool.tile([128, 17], f32, tag="sle0")
    slots_e1 = spool.tile([128, 17], f32, tag="sle1")
    slots_d0 = spool.tile([128, 17], f32, tag="sld0")
    slots_d1 = spool.tile([128, 17], f32, tag="sld1")
    zbuf = [zA, zB]

    # ---- init: z0 = sig0 * eps0^T ; slots_e[32] = sum 0.5 eps0^2 ----
    ep0 = epool.tile([128, BS], f32, tag="eps0t", bufs=1)
    nc.sync.dma_start(ep0[:], eps0T)
    nc.scalar.activation(zA[:], ep0[:], AF.Copy, bias=0.0, scale=tbl[:, 96:97])
    s0 = scrp.tile([128, 1], f32, tag="scrA")
    nc.scalar.activation(s0[:].broadcast_to((128, BS)), ep0[:], AF.Square,
                         bias=0.0, scale=SQ5, accum_out=slots_e0[:, 16:17])

    # ---- main loop (fully unrolled) ----
    for t in range(T):
        zin = zbuf[t % 2]
        zout = zbuf[(t + 1) % 2]
        ept = epool.tile([128, BS], f32r, tag="eps")
        nc.sync.dma_start(ept[:], epsT[t])
        z_r = hpool.tile([128, BS], f32r, tag="zr", bufs=2)
        for nn in range(NT):
            nc.gpsimd.tensor_copy(z_r[:, nn * 512:(nn + 1) * 512],
                                  zin[:, nn * 512:(nn + 1) * 512])

        for n in range(NT):
            nsl = slice(n * 512, (n + 1) * 512)
            zps = psZ.tile([128, 512], f32, tag="zps")
            for h in range(4):
                use_dve_c1 = (h, n) in KNOB["c1_dve"]
                hp = psH.tile([128, 512], f32, tag="hp")
                nc.tensor.matmul(hp[:], lhsT=w1z[:, h * 128:(h + 1) * 128],
                                 rhs=z_r[:, nsl], start=True,
                                 stop=(use_dve_c1 or "no_c1mm" in _ABLATE))
                if not use_dve_c1 and "no_c1mm" not in _ABLATE:
                    nc.tensor.matmul(hp[:], lhsT=ident[:], rhs=c1t[:, h, nsl],
                                     start=False, stop=True)
                hs = hpool.tile([128, 512], f32r, tag="hs")
                tecol = tet[:, h * 32 + t: h * 32 + t + 1]
                mw = 64 if "cheap_movers" in _ABLATE else 512
                if use_dve_c1:
                    tmp = hpool.tile([128, 512], f32, tag="tmpc1")
                    nc.vector.tensor_tensor(tmp[:, :mw], hp[:, :mw],
                                            c1t[:, h, nsl][:, :mw], op=ALU.add)
                    nc.vector.tensor_scalar(hs[:, :mw], tmp[:, :mw],
                                            scalar1=tecol, scalar2=0.0,
                                            op0=ALU.add, op1=ALU.max)
                elif h not in KNOB["dve_movers"]:
                    nc.scalar.activation(hs[:, :mw], hp[:, :mw], AF.Relu,
                                         bias=tecol, scale=1.0)
                else:
                    nc.vector.tensor_scalar(hs[:, :mw], hp[:, :mw], scalar1=tecol,
                                            scalar2=0.0, op0=ALU.add,
                                            op1=ALU.max)
                nc.tensor.matmul(zps[:], lhsT=w2t[:, h, :], rhs=hs[:],
                                 start=(h == 0),
                                 stop=(h == 3 and "no_diagmm" in _ABLATE))
            if "no_diagmm" not in _ABLATE:
                nc.tensor.matmul(zps[:], lhsT=diag[:, t, :], rhs=ept[:, nsl],
                                 start=False, stop=True)
            # z' = a_t * z + (u' + s_t eps)   [exact fp32 on DVE]
            nc.vector.scalar_tensor_tensor(
                zout[:, nsl], in0=zin[:, nsl], scalar=tbl[:, t:t + 1],
                in1=zps[:], op0=ALU.mult, op1=ALU.add)

        if "no_ew" in _ABLATE:
            continue
        # v = (z' * -c_t) + z          [DVE, exact fp32]
        v = scrp.tile([128, BS], f32, tag="scrV")
        nc.vector.scalar_tensor_tensor(
            v[:], in0=zout[:], scalar=tbl[:, 32 + t:33 + t], in1=zin[:],
            op0=ALU.mult, op1=ALU.add)
        # slots_d[t] = sum k_t v^2     [DVE STT with accum]
        sd = scrp.tile([128, 1], f32, tag="scrA")
        sdslot = (slots_d0 if t % 2 == 0 else slots_d1)[:, t // 2:t // 2 + 1]
        if KNOB["sqd_act"]:
            nc.scalar.activation(
                sd[:].broadcast_to((128, BS)), v[:], AF.Square, bias=0.0,
                scale=tbl[:, 97 + t:98 + t], accum_out=sdslot)
        else:
            nc.vector.scalar_tensor_tensor(
                sd[:].broadcast_to((128, BS)), in0=v[:],
                scalar=tbl[:, 64 + t:65 + t], in1=v[:],
                op0=ALU.mult, op1=ALU.mult, accum_out=sdslot)
        # slots_e[t] = sum 0.5 eps^2   [ACT Square accum]
        se = scrp.tile([128, 1], f32, tag="scrB")
        nc.scalar.activation(se[:].broadcast_to((128, BS)), ept[:], AF.Square,
                             bias=0.0, scale=SQ5,
                             accum_out=(slots_e0 if t % 2 == 0 else slots_e1)[:, t // 2:t // 2 + 1])

    # ---- terminal ----
    zfin = zbuf[T % 2]
    mu = epool.tile([128, BS], f32, tag="mut", bufs=1)
    nc.sync.dma_start(mu[:], muT)
    vT = scrp.tile([128, BS], f32, tag="scrV")
    nc.vector.tensor_sub(vT[:], zfin[:], mu[:])
    sT = scrp.tile([128, 1], f32, tag="scrA")
    nc.scalar.activation(sT[:].broadcast_to((128, BS)), vT[:], AF.Square,
                         bias=0.0, scale=SQ5, accum_out=slots_d0[:, 16:17])
    nc.gpsimd.memset(slots_e1[:, 16:17], 0.0)
    nc.gpsimd.memset(slots_d1[:, 16:17], 0.0)

    out4 = spool.tile([128, 4], f32, tag="out4")
    import concourse.mybir as mybir
    nc.vector.tensor_reduce(out4[:, 0:1], slots_e0[:],
                            axis=mybir.AxisListType.X, op=ALU.add)
    nc.vector.tensor_reduce(out4[:, 2:3], slots_e1[:],
                            axis=mybir.AxisListType.X, op=ALU.add)
    nc.vector.tensor_reduce(out4[:, 1:2], slots_d0[:],
                            axis=mybir.AxisListType.X, op=ALU.add)
    nc.vector.tensor_reduce(out4[:, 3:4], slots_d1[:],
                            axis=mybir.AxisListType.X, op=ALU.add)
    nc.sync.dma_start(outd, out4[:, 0:2])
    nc.sync.dma_start(outd2, out4[:, 2:4])


def _host_prep(inputs):
    """Numpy-only preprocessing: transposes, shards, scalar tables."""
    ctx = np.asarray(inputs["context_embedding"], np.float32)
    eps0 = np.asarray(inputs["eps0"], np.float32)
    eps = np.asarray(inputs["eps"], np.float32)
    beta = np.asarray(inputs["beta_schedule"], np.float64)
    sig0 = float(np.asarray(inputs["sigma0"], np.float32)[0])
    W1 = np.asarray(inputs["W1"], np.float32)
    b1 = np.asarray(inputs["b1"], np.float32)
    W2 = np.asarray(inputs["W2"], np.float32)
    b2 = np.asarray(inputs["b2"], np.float32)
    te = np.asarray(inputs["t_emb"], np.float32)
    mu = np.asarray(inputs["target_mu"], np.float32)

    dt = 1.0 / T
    bb = np.roll(beta, 1)
    a_t = (1.0 + beta * dt).astype(np.float32)
    c_t = (1.0 - bb * dt).astype(np.float32)
    s_t = (np.sqrt(2.0 * beta * dt) * sig0).astype(np.float32)
    sb_t = (np.sqrt(2.0 * bb * dt) * sig0).astype(np.float32)
    k_t = (0.5 / (sb_t.astype(np.float64) ** 2)).astype(np.float32)

    if np.any(b2):
        raise NotImplementedError("nonzero b2 not supported by this kernel")

    c1 = (ctx @ W1[Z:] + b1).astype(np.float32)          # [B, HID]
    c1_T = np.ascontiguousarray(c1.T)                     # [HID, B]
    eps_T = np.ascontiguousarray(eps.transpose(0, 2, 1))  # [T, Z, B]
    eps0_T = np.ascontiguousarray(eps0.T)                 # [Z, B]
    mu_T = np.ascontiguousarray(mu.T)                     # [Z, B]

    w1zd = np.ascontiguousarray(W1[:Z])                   # [Z, HID]
    w2d = np.ascontiguousarray(W2 * np.float32(dt))       # [HID, Z]
    identd = np.eye(128, dtype=np.float32)
    diagd = np.zeros((T, 128, 128), np.float32)
    idx = np.arange(128)
    for t in range(T):
        diagd[t, idx, idx] = s_t[t]

    tbl = np.zeros((128, 160), np.float32)
    tbl[:, 0:T] = a_t[None, :]
    tbl[:, 32:32 + T] = -c_t[None, :]
    tbl[:, 64:64 + T] = k_t[None, :]
    tbl[:, 96] = sig0
    tbl[:, 97:97 + T] = np.sqrt(k_t.astype(np.float64)).astype(np.float32)[None, :]

    # b1 is already folded into c1; the bias table is te alone.
    tet = np.zeros((128, 128), np.float32)
    for h in range(4):
        tet[:, h * 32:(h + 1) * 32] = te[:, h * 128:(h + 1) * 128].T

    const = float(np.sum(np.log(s_t.astype(np.float64))
                         - np.log(sb_t.astype(np.float64))) + math.log(sig0))

    in_maps = []
    for c in range(NCORES):
        bs = slice(c * BS, (c + 1) * BS)
        in_maps.append({
            "epsT": np.ascontiguousarray(eps_T[:, :, bs]),
            "eps0T": np.ascontiguousarray(eps0_T[:, bs]),
            "muT": np.ascontiguousarray(mu_T[:, bs]),
            "c1d": np.ascontiguousarray(c1_T.reshape(4, 128, B)[:, :, bs]),
            "w1zd": w1zd,
            "w2d": w2d,
            "identd": identd,
            "diagd": diagd,
            "tbld": tbl,
            "tetd": tet,
        })
    return in_maps, const


def _install_neff_cache():
    """Cache walrus NEFF output by BIR hash (compile takes ~6 min otherwise)."""
    import hashlib
    import os
    import shutil

    from concourse import bass2jax

    if getattr(bass2jax, "_ant_neff_cache_installed", False):
        return
    orig = bass2jax.compile_bir_kernel
    cache_dir = os.environ.get("BASS_NEFF_CACHE", "/tmp/neff_cache")

    def cached(bir_json, tmpdir, neff_name="file.neff"):
        os.makedirs(cache_dir, exist_ok=True)
        key = hashlib.sha256(bir_json if isinstance(bir_json, bytes)
                             else bir_json.encode()).hexdigest()[:24]
        hit = os.path.join(cache_dir, f"{key}.neff")
        dst = os.path.join(tmpdir, neff_name)
        if os.path.exists(hit):
            shutil.copy(hit, dst)
            return dst
        out = orig(bir_json, tmpdir, neff_name)
        shutil.copy(out, hit)
        return out

    bass2jax.compile_bir_kernel = cached
    bass2jax._ant_neff_cache_installed = True


def kernel(**inputs) -> np.ndarray:
    from concourse import bass_utils

    _install_neff_cache()
    if "nc" not in _cache:
        _cache["nc"] = _build_module()
    nc = _cache["nc"]

    in_maps, const = _host_prep(inputs)
    res = bass_utils.run_bass_kernel_spmd(nc, in_maps, core_ids=list(range(NCORES)))
    _cache["last_res"] = res
    total = 0.0
    for c in range(NCORES):
        o = res.results[c]["outd"].astype(np.float64)
        o2 = res.results[c]["outd2"].astype(np.float64)
        total += float(np.sum(o[:, 0] - o[:, 1] + o2[:, 0] - o2[:, 1]))
    total = total / B + Z * const
    return np.float32(total)

